# revision 5
# baseline (speedup 1.0000x reference)
"""2-layer GAT (100000 nodes, 32 neighbors) on 8 trn2 NeuronCores.

Strategy (SPMD, one Bass program for all 8 cores; nodes sharded 8 ways):
  - Layer 1 needs NO on-device gather: neighbor indices are static inputs,
    so the host pre-expands node features into edge order (one 128-column
    block per destination tile and neighbor slot, plus the tile's own
    columns).  Each tile then runs 33 matmuls against a fused rhs
    [W1 | W1@A1blk | W1@A2blk] emitting h1/s1/s2 per edge directly.
  - Attention (leaky_relu -> exp -> softmax -> weighted sum) on DVE/ACT in
    node-per-partition layout; ELU; PE-transpose feeds the layer-2 matmul
    against [W2 | W2@A1blk2 | W2@A2blk2] per tile, producing each core's
    shard of the layer-2 node table [h2(128)|s1(8)|s2(8)] in bf16.
  - Table shards are AllGathered (28.9 MB) so every core sees all rows;
    layer-2 neighbor rows are fetched with per-slot indirect DMAs
    (one [128,1]-offset gather per neighbor slot per tile).
  - Layer-2 attention, head-mean and softmax produce the output shard.
Host remaps neighbor ids to the partition-major table layout and
de-permutes the output shard.
"""
import sys

if '/opt/trn_rl_repo' not in sys.path:
    sys.path.insert(0, '/opt/trn_rl_repo')

import numpy as np
import ml_dtypes
import concourse.bass as bass
import concourse.bacc as bacc
import concourse.mybir as mybir
from concourse.tile import TileContext
from concourse.masks import make_identity

import jax
from jax.sharding import Mesh, PartitionSpec
from jax.experimental.shard_map import shard_map
from concourse.bass2jax import (_bass_exec_p, install_neuronx_cc_hook,
                                partition_id_tensor)

FP = mybir.dt.float32
BF = mybir.dt.bfloat16
I32 = mybir.dt.int32
AF = mybir.ActivationFunctionType
OP = mybir.AluOpType
AX = mybir.AxisListType

N_NODES = 100000
N_CORES = 8
D_NBR = 32
K1, F1 = 8, 8
K2, F2 = 8, 16
NEG_SLOPE = 0.01
S = N_NODES // N_CORES          # 12500
NT = (S + 127) // 128           # 98 tiles
SP = NT * 128                   # 12544 padded shard rows
H1, H2 = K1 * F1, K2 * F2       # 64, 128
R2 = H2 + 16                    # 144 bf16 = 288B table-2 row
R2F = R2 // 2                   # 72 f32 (fp32-typed view for the gather)
EC = (D_NBR + 1) * 128          # 4224 edge+own columns per tile


def _build_gat():
    nc = bacc.Bacc("TRN2", target_bir_lowering=False, debug=False,
                   num_devices=N_CORES)
    xe = nc.dram_tensor("xe", [128, NT * EC], BF, kind="ExternalInput").ap()
    rt1d = nc.dram_tensor("rt1", [128, H1 + 16], BF, kind="ExternalInput").ap()
    rt2d = nc.dram_tensor("rt2", [H1, R2], BF, kind="ExternalInput").ap()
    idx2 = nc.dram_tensor("idx2", [128, NT * D_NBR], I32,
                          kind="ExternalInput").ap()
    out = nc.dram_tensor("out", [128, NT * F2], FP, kind="ExternalOutput").ap()

    t2m = nc.dram_tensor("t2m", [SP, R2F], FP).ap()
    t2f = nc.dram_tensor("t2f", [N_CORES * SP, R2F], FP,
                         addr_space="Shared").ap()

    with TileContext(nc) as tc:
        with tc.tile_pool(name="const", bufs=1) as cpool, \
             tc.tile_pool(name="xep", bufs=3) as xep, \
             tc.tile_pool(name="att", bufs=2) as ap_, \
             tc.tile_pool(name="ph", bufs=4, space="PSUM") as php, \
             tc.tile_pool(name="pmisc", bufs=2, space="PSUM") as pmp, \
             tc.tile_pool(name="ptr", bufs=1, space="PSUM") as ptp, \
             tc.tile_pool(name="pt2", bufs=1, space="PSUM") as pt2p:

            rt1 = cpool.tile([128, H1 + 16], BF)
            nc.sync.dma_start(out=rt1[:], in_=rt1d[:, :])
            rt2 = cpool.tile([H1, R2], BF)
            nc.sync.dma_start(out=rt2[:], in_=rt2d[:, :])
            ident = cpool.tile([128, 128], BF)
            make_identity(nc, ident[:])
            idxs = cpool.tile([128, NT * D_NBR], I32)
            nc.sync.dma_start(out=idxs[:], in_=idx2[:, :])
            t2rows = cpool.tile([128, NT * R2], BF)
            outacc = cpool.tile([128, NT * F2], FP)

            # ---------------- layer 1 (edge-streamed) + layer-2 table ----
            for t in range(NT):
                xet = xep.tile([128, EC], BF, name=f"xe{t}", tag="xe")
                nc.sync.dma_start(out=xet[:], in_=xe[:, t * EC:(t + 1) * EC])

                hg = ap_.tile([128, D_NBR * (H1 + 16)], BF,
                              name=f"hg{t}", tag="hg")
                hgv = hg[:].rearrange("p (d r) -> p d r", r=H1 + 16)
                for g in range(8):           # 8 groups of 4 neighbor slots
                    ps = php.tile([128, 512], FP, name=f"ph{t}_{g}", tag="ph",
                                  space="PSUM")
                    for s in range(4):
                        d = g * 4 + s
                        nc.tensor.matmul(
                            out=ps[:, s * 128:s * 128 + H1 + 16],
                            lhsT=xet[:, d * 128:(d + 1) * 128], rhs=rt1[:],
                            start=True, stop=True)
                    pv = ps[:].rearrange("p (s r) -> p s r", r=128)
                    if g % 2 == 0:
                        nc.vector.tensor_copy(
                            out=hgv[:, g * 4:(g + 1) * 4, :],
                            in_=pv[:, :, :H1 + 16])
                    else:
                        nc.scalar.activation(
                            out=hgv[:, g * 4:(g + 1) * 4, :],
                            in_=pv[:, :, :H1 + 16], func=AF.Copy)
                pown = pmp.tile([128, H1 + 16], FP, name=f"po{t}", tag="po",
                                space="PSUM")
                nc.tensor.matmul(out=pown[:], lhsT=xet[:, D_NBR * 128:],
                                 rhs=rt1[:], start=True, stop=True)
                s1t = ap_.tile([128, K1], BF, name=f"s1{t}", tag="s1")
                nc.vector.tensor_copy(out=s1t[:], in_=pown[:, H1:H1 + 8])

                # attention scores: e = s1_i + s2_j ; exp(leaky_relu(e))
                e = ap_.tile([128, D_NBR * K1], BF, name=f"e{t}", tag="e")
                nc.vector.tensor_tensor(
                    out=e[:].rearrange("p (d k) -> p d k", k=K1),
                    in0=hgv[:, :, H1 + 8:H1 + 16],
                    in1=s1t[:].unsqueeze(1).to_broadcast([128, D_NBR, K1]),
                    op=OP.add)
                u = ap_.tile([128, D_NBR * K1], BF, name=f"u{t}", tag="u")
                nc.scalar.activation(out=u[:], in_=e[:], func=AF.Lrelu,
                                     alpha=NEG_SLOPE)
                nc.scalar.activation(out=u[:], in_=u[:], func=AF.Exp)
                z = ap_.tile([128, K1], FP, name=f"z{t}", tag="z")
                nc.vector.tensor_reduce(
                    out=z[:],
                    in_=u[:].rearrange("p (d k) -> p d k", k=K1)
                        .transpose([0, 2, 1]),
                    axis=AX.X, op=OP.add)
                rz = ap_.tile([128, K1], FP, name=f"rz{t}", tag="rz")
                nc.vector.reciprocal(out=rz[:], in_=z[:])
                tmp = ap_.tile([128, H1 * D_NBR], BF, name=f"tm{t}", tag="tm")
                h4 = hgv[:, :, 0:H1].rearrange("p d (k f) -> p d k f", f=F1) \
                    .transpose([0, 2, 3, 1])
                u4 = u[:].rearrange("p (d k) -> p d k", k=K1).unsqueeze(3) \
                    .to_broadcast([128, D_NBR, K1, F1]).transpose([0, 2, 3, 1])
                nc.vector.tensor_tensor(
                    out=tmp[:].rearrange("p (k f d) -> p k f d",
                                         f=F1, d=D_NBR),
                    in0=h4, in1=u4, op=OP.mult)
                sw = ap_.tile([128, H1], FP, name=f"sw{t}", tag="sw")
                nc.vector.tensor_reduce(
                    out=sw[:],
                    in_=tmp[:].rearrange("p (kf d) -> p kf d", d=D_NBR),
                    axis=AX.X, op=OP.add)
                o = ap_.tile([128, H1], FP, name=f"o{t}", tag="o")
                nc.vector.tensor_tensor(
                    out=o[:].rearrange("p (k f) -> p k f", f=F1),
                    in0=sw[:].rearrange("p (k f) -> p k f", f=F1),
                    in1=rz[:].unsqueeze(2).to_broadcast([128, K1, F1]),
                    op=OP.mult)
                # elu(o) = max(o, exp(min(o,0)) - 1), cast to bf16
                mn = ap_.tile([128, H1], FP, name=f"mn{t}", tag="mn")
                nc.vector.tensor_scalar_min(out=mn[:], in0=o[:], scalar1=0.0)
                nc.scalar.activation(out=mn[:], in_=mn[:], func=AF.Exp)
                x2 = ap_.tile([128, H1], BF, name=f"x2{t}", tag="x2")
                nc.vector.scalar_tensor_tensor(
                    out=x2[:], in0=mn[:], scalar=-1.0, in1=o[:],
                    op0=OP.add, op1=OP.max)
                # layer-2 table rows for own nodes
                ptr = ptp.tile([H1, 128], BF, name=f"pt{t}", tag="pt",
                               space="PSUM")
                nc.tensor.transpose(out=ptr[:], in_=x2[:], identity=ident[:])
                x2T = ap_.tile([H1, 128], BF, name=f"xt{t}", tag="xt")
                nc.scalar.activation(out=x2T[:], in_=ptr[:], func=AF.Copy)
                pt2 = pt2p.tile([128, R2], FP, name=f"p2{t}", tag="p2",
                                space="PSUM")
                nc.tensor.matmul(out=pt2[:], lhsT=x2T[:], rhs=rt2[:],
                                 start=True, stop=True)
                nc.vector.tensor_copy(
                    out=t2rows[:, t * R2:(t + 1) * R2], in_=pt2[:])

            # store shard (partition-major rows: row = p*NT + t) + AllGather
            nc.sync.dma_start(
                out=t2m[:, :].rearrange("(p t) r -> p (t r)", p=128),
                in_=t2rows[:].bitcast(FP))
            nc.gpsimd.collective_compute(
                "AllGather", OP.bypass,
                replica_groups=[list(range(N_CORES))],
                ins=[t2m.opt()], outs=[t2f.opt()])

            # ---------------- layer 2 attention ---------------------------
            for t in range(NT):
                hg2 = ap_.tile([128, D_NBR * R2F], FP, name=f"g2{t}", tag="g2")
                hv = hg2[:].rearrange("p (d r) -> p d r", r=R2F)
                for j in range(D_NBR):
                    nc.gpsimd.indirect_dma_start(
                        out=hv[:, j, :], out_offset=None, in_=t2f[:],
                        in_offset=bass.IndirectOffsetOnAxis(
                            ap=idxs[:, t * D_NBR + j:t * D_NBR + j + 1],
                            axis=0))
                hb = hg2[:].bitcast(BF).rearrange("p (d r) -> p d r", r=R2)
                e2 = ap_.tile([128, D_NBR * K2], BF, name=f"e2{t}", tag="e2")
                nc.vector.tensor_tensor(
                    out=e2[:].rearrange("p (d k) -> p d k", k=K2),
                    in0=hb[:, :, H2 + 8:H2 + 16],
                    in1=t2rows[:, t * R2 + H2:t * R2 + H2 + 8].unsqueeze(1)
                        .to_broadcast([128, D_NBR, K2]),
                    op=OP.add)
                u2 = ap_.tile([128, D_NBR * K2], BF, name=f"u2{t}", tag="u2")
                nc.scalar.activation(out=u2[:], in_=e2[:], func=AF.Lrelu,
                                     alpha=NEG_SLOPE)
                nc.scalar.activation(out=u2[:], in_=u2[:], func=AF.Exp)
                z2 = ap_.tile([128, K2], FP, name=f"z2{t}", tag="z2")
                nc.vector.tensor_reduce(
                    out=z2[:],
                    in_=u2[:].rearrange("p (d k) -> p d k", k=K2)
                        .transpose([0, 2, 1]),
                    axis=AX.X, op=OP.add)
                rz2 = ap_.tile([128, K2], FP, name=f"rz2{t}", tag="rz2")
                nc.vector.reciprocal(out=rz2[:], in_=z2[:])
                tmp2 = ap_.tile([128, H2 * D_NBR], BF, name=f"t2{t}", tag="t2")
                h24 = hb[:, :, 0:H2].rearrange("p d (k f) -> p d k f", f=F2) \
                    .transpose([0, 2, 3, 1])
                u24 = u2[:].rearrange("p (d k) -> p d k", k=K2).unsqueeze(3) \
                    .to_broadcast([128, D_NBR, K2, F2]).transpose([0, 2, 3, 1])
                nc.vector.tensor_tensor(
                    out=tmp2[:].rearrange("p (k f d) -> p k f d",
                                          f=F2, d=D_NBR),
                    in0=h24, in1=u24, op=OP.mult)
                s2t = ap_.tile([128, H2], FP, name=f"s2{t}", tag="s2")
                nc.vector.tensor_reduce(
                    out=s2t[:],
                    in_=tmp2[:].rearrange("p (kf d) -> p kf d", d=D_NBR),
                    axis=AX.X, op=OP.add)
                o2 = ap_.tile([128, H2], FP, name=f"o2{t}", tag="o2")
                nc.vector.tensor_tensor(
                    out=o2[:].rearrange("p (k f) -> p k f", f=F2),
                    in0=s2t[:].rearrange("p (k f) -> p k f", f=F2),
                    in1=rz2[:].unsqueeze(2).to_broadcast([128, K2, F2]),
                    op=OP.mult)
                mo = ap_.tile([128, F2], FP, name=f"mo{t}", tag="mo")
                nc.vector.tensor_reduce(
                    out=mo[:],
                    in_=o2[:].rearrange("p (k f) -> p k f", f=F2)
                        .transpose([0, 2, 1]),
                    axis=AX.X, op=OP.add)
                u3 = ap_.tile([128, F2], FP, name=f"u3{t}", tag="u3")
                z3 = ap_.tile([128, 1], FP, name=f"z3{t}", tag="z3")
                nc.scalar.activation(out=u3[:], in_=mo[:], func=AF.Exp,
                                     scale=1.0 / K2, accum_out=z3[:])
                rz3 = ap_.tile([128, 1], FP, name=f"rz3{t}", tag="rz3")
                nc.vector.reciprocal(out=rz3[:], in_=z3[:])
                nc.vector.tensor_tensor(
                    out=outacc[:, t * F2:(t + 1) * F2], in0=u3[:],
                    in1=rz3[:].to_broadcast([128, F2]), op=OP.mult)

            nc.sync.dma_start(out=out[:, :], in_=outacc[:])

    nc.finalize()
    return nc


class _SpmdRunner:
    """jit-once SPMD executor over the 8 axon NeuronCores."""

    def __init__(self, nc, n_cores):
        install_neuronx_cc_hook()
        self.nc, self.n_cores = nc, n_cores
        partition_name = (nc.partition_id_tensor.name
                          if nc.partition_id_tensor else None)
        in_names, out_names, out_avals, zero_outs = [], [], [], []
        for alloc in nc.m.functions[0].allocations:
            if not isinstance(alloc, mybir.MemoryLocationSet):
                continue
            name = alloc.memorylocations[0].name
            if alloc.kind == "ExternalInput":
                if name != partition_name:
                    in_names.append(name)
            elif alloc.kind == "ExternalOutput":
                out_names.append(name)
                shape = tuple(alloc.tensor_shape)
                dtype = mybir.dt.np(alloc.dtype)
                out_avals.append(jax.core.ShapedArray(shape, dtype))
                zero_outs.append(np.zeros(shape, dtype))
        self.in_names, self.out_names = in_names, out_names
        self.out_avals, self.zero_outs = out_avals, zero_outs
        all_in_names = in_names + out_names
        if partition_name is not None:
            all_in_names.append(partition_name)

        def _body(*args):
            operands = list(args)
            if partition_name is not None:
                operands.append(partition_id_tensor())
            return tuple(_bass_exec_p.bind(
                *operands, out_avals=tuple(out_avals),
                in_names=tuple(all_in_names), out_names=tuple(out_names),
                lowering_input_output_aliases=(),
                sim_require_finite=True, sim_require_nnan=True, nc=nc))

        devices = jax.devices()[:n_cores]
        self.mesh = Mesh(np.asarray(devices), ("core",))
        n_params, n_outs = len(in_names), len(out_avals)
        in_specs = (PartitionSpec("core"),) * (n_params + n_outs)
        out_specs = (PartitionSpec("core"),) * n_outs
        self.fn = jax.jit(
            shard_map(_body, mesh=self.mesh, in_specs=in_specs,
                      out_specs=out_specs, check_rep=False),
            keep_unused=True)
        self.sharding = jax.sharding.NamedSharding(self.mesh,
                                                   PartitionSpec("core"))

    def run(self, in_maps):
        per_core = [[np.asarray(m[n]) for n in self.in_names] for m in in_maps]
        concat = [np.concatenate([per_core[c][i] for c in range(self.n_cores)],
                                 axis=0) for i in range(len(self.in_names))]
        zeros = [np.zeros((self.n_cores * z.shape[0], *z.shape[1:]), z.dtype)
                 for z in self.zero_outs]
        dev = [jax.device_put(a, self.sharding) for a in concat + zeros]
        outs = self.fn(*dev)
        jax.block_until_ready(outs)
        res = []
        for c in range(self.n_cores):
            res.append({name: np.asarray(outs[i]).reshape(
                self.n_cores, *self.out_avals[i].shape)[c]
                for i, name in enumerate(self.out_names)})
        return res


def _host_prep(node_features, neighbors, W1, a1_1, a2_1, W2, a1_2, a2_2):
    def blk(a, k, f):
        A = np.zeros((k * f, k), np.float32)
        for kk in range(k):
            A[kk * f:(kk + 1) * f, kk] = a[kk]
        return A

    rt1 = np.concatenate(
        [W1, W1 @ blk(a1_1, K1, F1), W1 @ blk(a2_1, K1, F1)],
        axis=1).astype(ml_dtypes.bfloat16)
    rt2 = np.concatenate(
        [W2, W2 @ blk(a1_2, K2, F2), W2 @ blk(a2_2, K2, F2)],
        axis=1).astype(ml_dtypes.bfloat16)
    x_bf = node_features.astype(ml_dtypes.bfloat16)
    nbr = neighbors.astype(np.int64)

    # map a global node id to its padded table-row id (r*SP + p*NT + t)
    g = np.arange(N_NODES, dtype=np.int64)
    r_, w_ = g // S, g % S
    rowmap = (r_ * SP + (w_ % 128) * NT + w_ // 128).astype(np.int32)

    in_maps = []
    for r in range(N_CORES):
        own = np.arange(r * S, (r + 1) * S, dtype=np.int64)
        own = np.concatenate([own, np.full(SP - S, r * S, np.int64)])
        own_pt = own.reshape(NT, 128)                    # [t, p]
        nb = nbr[own]                                    # [SP, 32]
        nb_pt = nb.reshape(NT, 128, D_NBR)               # [t, p, j]
        # edge stream: per tile, columns (d*128+p) then own (p)
        eidx = np.concatenate(
            [nb_pt.transpose(0, 2, 1),                   # [t, d, p]
             own_pt[:, None, :]], axis=1)                # [t, 33, p]
        xeT = np.ascontiguousarray(
            x_bf[eidx.reshape(-1)].T)                    # [128, NT*EC]
        idx2 = np.ascontiguousarray(
            rowmap[nb_pt].transpose(1, 0, 2)             # [p, t, j]
            .reshape(128, NT * D_NBR))
        in_maps.append({'xe': xeT, 'rt1': rt1, 'rt2': rt2, 'idx2': idx2})
    return in_maps


_RUNNER = None


def _get_runner():
    global _RUNNER
    if _RUNNER is None:
        nc = _build_gat()
        _RUNNER = _SpmdRunner(nc, N_CORES)
    return _RUNNER


def kernel(node_features, neighbors, W1, a1_1, a2_1, W2, a1_2, a2_2):
    node_features = np.asarray(node_features, dtype=np.float32)
    runner = _get_runner()
    in_maps = _host_prep(node_features, np.asarray(neighbors),
                         np.asarray(W1, np.float32),
                         np.asarray(a1_1, np.float32),
                         np.asarray(a2_1, np.float32),
                         np.asarray(W2, np.float32),
                         np.asarray(a1_2, np.float32),
                         np.asarray(a2_2, np.float32))
    res = runner.run(in_maps)
    parts = []
    for c in range(N_CORES):
        o = res[c]['out'].reshape(128, NT, F2).transpose(1, 0, 2)
        parts.append(o.reshape(SP, F2)[:S])
    return np.concatenate(parts, axis=0).astype(np.float32)


# revision 7
# speedup vs baseline: 1.4816x; 1.4816x over previous
"""2-layer GAT (100000 nodes, 32 neighbors) on 8 trn2 NeuronCores.

Strategy (SPMD, one Bass program for all 8 cores; nodes sharded 8 ways):
  - Layer 1 needs NO on-device gather: neighbor indices are static inputs,
    so the host pre-expands node features into edge order (one 128-column
    block per destination tile and neighbor slot, plus the tile's own
    columns).  Each tile then runs 33 matmuls against a fused rhs
    [W1 | W1@A1blk | W1@A2blk] emitting h1/s1/s2 per edge directly.
  - Attention (leaky_relu -> exp -> softmax -> weighted sum) on DVE/ACT in
    node-per-partition layout; ELU; PE-transpose feeds the layer-2 matmul
    against [W2 | W2@A1blk2 | W2@A2blk2] per tile, producing each core's
    shard of the layer-2 node table [h2(128)|s1(8)|s2(8)] in bf16.
  - Table shards are AllGathered (28.9 MB) so every core sees all rows;
    layer-2 neighbor rows are fetched with per-slot indirect DMAs
    (one [128,1]-offset gather per neighbor slot per tile).
  - Layer-2 attention, head-mean and softmax produce the output shard.
Host remaps neighbor ids to the partition-major table layout and
de-permutes the output shard.
"""
import sys

if '/opt/trn_rl_repo' not in sys.path:
    sys.path.insert(0, '/opt/trn_rl_repo')

import numpy as np
import ml_dtypes
import concourse.bass as bass
import concourse.bacc as bacc
import concourse.mybir as mybir
from concourse.tile import TileContext
from concourse.masks import make_identity

import jax
from jax.sharding import Mesh, PartitionSpec
from jax.experimental.shard_map import shard_map
from concourse.bass2jax import (_bass_exec_p, install_neuronx_cc_hook,
                                partition_id_tensor)

FP = mybir.dt.float32
BF = mybir.dt.bfloat16
I32 = mybir.dt.int32
AF = mybir.ActivationFunctionType
OP = mybir.AluOpType
AX = mybir.AxisListType

N_NODES = 100000
N_CORES = 8
D_NBR = 32
K1, F1 = 8, 8
K2, F2 = 8, 16
NEG_SLOPE = 0.01
S = N_NODES // N_CORES          # 12500
NT = (S + 127) // 128           # 98 tiles
SP = NT * 128                   # 12544 padded shard rows
H1, H2 = K1 * F1, K2 * F2       # 64, 128
R2 = H2 + 16                    # 144 bf16 = 288B table-2 row
R2F = R2 // 2                   # 72 f32 (fp32-typed view for the gather)
EC = (D_NBR + 1) * 128          # 4224 edge+own columns per tile


def _build_gat():
    nc = bacc.Bacc("TRN2", target_bir_lowering=False, debug=False,
                   num_devices=N_CORES)
    xe = nc.dram_tensor("xe", [128, NT * EC], BF, kind="ExternalInput").ap()
    rt1d = nc.dram_tensor("rt1", [128, H1 + 16], BF, kind="ExternalInput").ap()
    rt2d = nc.dram_tensor("rt2", [H1, R2], BF, kind="ExternalInput").ap()
    idx2 = nc.dram_tensor("idx2", [128, NT * D_NBR], I32,
                          kind="ExternalInput").ap()
    out = nc.dram_tensor("out", [128, NT * F2], FP, kind="ExternalOutput").ap()

    t2m = nc.dram_tensor("t2m", [SP, R2F], FP).ap()
    t2f = nc.dram_tensor("t2f", [N_CORES * SP, R2F], FP,
                         addr_space="Shared").ap()

    with TileContext(nc) as tc:
        with tc.tile_pool(name="const", bufs=1) as cpool, \
             tc.tile_pool(name="xep", bufs=3) as xep, \
             tc.tile_pool(name="att", bufs=2) as ap_, \
             tc.tile_pool(name="g2p", bufs=4) as g2p, \
             tc.tile_pool(name="ph", bufs=4, space="PSUM") as php, \
             tc.tile_pool(name="pmisc", bufs=2, space="PSUM") as pmp, \
             tc.tile_pool(name="ptr", bufs=1, space="PSUM") as ptp, \
             tc.tile_pool(name="pt2", bufs=1, space="PSUM") as pt2p:

            rt1 = cpool.tile([128, H1 + 16], BF)
            nc.sync.dma_start(out=rt1[:], in_=rt1d[:, :])
            rt2 = cpool.tile([H1, R2], BF)
            nc.sync.dma_start(out=rt2[:], in_=rt2d[:, :])
            ident = cpool.tile([128, 128], BF)
            make_identity(nc, ident[:])
            idxs = cpool.tile([128, NT * D_NBR], I32)
            nc.sync.dma_start(out=idxs[:], in_=idx2[:, :])
            t2rows = cpool.tile([128, NT * R2], BF)
            outacc = cpool.tile([128, NT * F2], FP)

            # ---------------- layer 1 (edge-streamed) + layer-2 table ----
            for t in range(NT):
                xet = xep.tile([128, EC], BF, name=f"xe{t}", tag="xe")
                nc.sync.dma_start(out=xet[:], in_=xe[:, t * EC:(t + 1) * EC])

                hg = ap_.tile([128, D_NBR * (H1 + 16)], BF,
                              name=f"hg{t}", tag="hg")
                hgv = hg[:].rearrange("p (d r) -> p d r", r=H1 + 16)
                for g in range(8):           # 8 groups of 4 neighbor slots
                    ps = php.tile([128, 512], FP, name=f"ph{t}_{g}", tag="ph",
                                  space="PSUM")
                    for s in range(4):
                        d = g * 4 + s
                        nc.tensor.matmul(
                            out=ps[:, s * 128:s * 128 + H1 + 16],
                            lhsT=xet[:, d * 128:(d + 1) * 128], rhs=rt1[:],
                            start=True, stop=True)
                    pv = ps[:].rearrange("p (s r) -> p s r", r=128)
                    if g % 2 == 0:
                        nc.vector.tensor_copy(
                            out=hgv[:, g * 4:(g + 1) * 4, :],
                            in_=pv[:, :, :H1 + 16])
                    else:
                        nc.scalar.activation(
                            out=hgv[:, g * 4:(g + 1) * 4, :],
                            in_=pv[:, :, :H1 + 16], func=AF.Copy)
                pown = pmp.tile([128, H1 + 16], FP, name=f"po{t}", tag="po",
                                space="PSUM")
                nc.tensor.matmul(out=pown[:], lhsT=xet[:, D_NBR * 128:],
                                 rhs=rt1[:], start=True, stop=True)
                s1t = ap_.tile([128, K1], BF, name=f"s1{t}", tag="s1")
                nc.vector.tensor_copy(out=s1t[:], in_=pown[:, H1:H1 + 8])

                # attention scores: e = s1_i + s2_j ; exp(leaky_relu(e))
                e = ap_.tile([128, D_NBR * K1], BF, name=f"e{t}", tag="e")
                nc.vector.tensor_tensor(
                    out=e[:].rearrange("p (d k) -> p d k", k=K1),
                    in0=hgv[:, :, H1 + 8:H1 + 16],
                    in1=s1t[:].unsqueeze(1).to_broadcast([128, D_NBR, K1]),
                    op=OP.add)
                u = ap_.tile([128, D_NBR * K1], BF, name=f"u{t}", tag="u")
                nc.scalar.activation(out=u[:], in_=e[:], func=AF.Lrelu,
                                     alpha=NEG_SLOPE)
                nc.scalar.activation(out=u[:], in_=u[:], func=AF.Exp)
                z = ap_.tile([128, K1], FP, name=f"z{t}", tag="z")
                nc.vector.tensor_reduce(
                    out=z[:],
                    in_=u[:].rearrange("p (d k) -> p d k", k=K1)
                        .transpose([0, 2, 1]),
                    axis=AX.X, op=OP.add)
                rz = ap_.tile([128, K1], FP, name=f"rz{t}", tag="rz")
                nc.vector.reciprocal(out=rz[:], in_=z[:])
                tmp = ap_.tile([128, H1 * D_NBR], BF, name=f"tm{t}", tag="tm")
                h4 = hgv[:, :, 0:H1].rearrange("p d (k f) -> p d k f", f=F1) \
                    .transpose([0, 2, 3, 1])
                u4 = u[:].rearrange("p (d k) -> p d k", k=K1).unsqueeze(3) \
                    .to_broadcast([128, D_NBR, K1, F1]).transpose([0, 2, 3, 1])
                nc.vector.tensor_tensor(
                    out=tmp[:].rearrange("p (k f d) -> p k f d",
                                         f=F1, d=D_NBR),
                    in0=h4, in1=u4, op=OP.mult)
                sw = ap_.tile([128, H1], FP, name=f"sw{t}", tag="sw")
                nc.vector.tensor_reduce(
                    out=sw[:],
                    in_=tmp[:].rearrange("p (kf d) -> p kf d", d=D_NBR),
                    axis=AX.X, op=OP.add)
                o = ap_.tile([128, H1], FP, name=f"o{t}", tag="o")
                nc.vector.tensor_tensor(
                    out=o[:].rearrange("p (k f) -> p k f", f=F1),
                    in0=sw[:].rearrange("p (k f) -> p k f", f=F1),
                    in1=rz[:].unsqueeze(2).to_broadcast([128, K1, F1]),
                    op=OP.mult)
                # elu(o) = max(o, exp(min(o,0)) - 1), cast to bf16
                mn = ap_.tile([128, H1], FP, name=f"mn{t}", tag="mn")
                nc.vector.tensor_scalar_min(out=mn[:], in0=o[:], scalar1=0.0)
                nc.scalar.activation(out=mn[:], in_=mn[:], func=AF.Exp)
                x2 = ap_.tile([128, H1], BF, name=f"x2{t}", tag="x2")
                nc.vector.scalar_tensor_tensor(
                    out=x2[:], in0=mn[:], scalar=-1.0, in1=o[:],
                    op0=OP.add, op1=OP.max)
                # layer-2 table rows for own nodes
                ptr = ptp.tile([H1, 128], BF, name=f"pt{t}", tag="pt",
                               space="PSUM")
                nc.tensor.transpose(out=ptr[:], in_=x2[:], identity=ident[:])
                x2T = ap_.tile([H1, 128], BF, name=f"xt{t}", tag="xt")
                nc.scalar.activation(out=x2T[:], in_=ptr[:], func=AF.Copy)
                pt2 = pt2p.tile([128, R2], FP, name=f"p2{t}", tag="p2",
                                space="PSUM")
                nc.tensor.matmul(out=pt2[:], lhsT=x2T[:], rhs=rt2[:],
                                 start=True, stop=True)
                nc.vector.tensor_copy(
                    out=t2rows[:, t * R2:(t + 1) * R2], in_=pt2[:])

            # store shard (partition-major rows: row = p*NT + t) + AllGather
            nc.sync.dma_start(
                out=t2m[:, :].rearrange("(p t) r -> p (t r)", p=128),
                in_=t2rows[:].bitcast(FP))
            nc.gpsimd.collective_compute(
                "AllGather", OP.bypass,
                replica_groups=[list(range(N_CORES))],
                ins=[t2m.opt()], outs=[t2f.opt()])

            # ---------------- layer 2 attention ---------------------------
            for t in range(NT):
                hg2 = g2p.tile([128, D_NBR * R2F], FP, name=f"g2{t}", tag="g2")
                hv = hg2[:].rearrange("p (d r) -> p d r", r=R2F)
                for j in range(D_NBR):
                    nc.gpsimd.indirect_dma_start(
                        out=hv[:, j, :], out_offset=None, in_=t2f[:],
                        in_offset=bass.IndirectOffsetOnAxis(
                            ap=idxs[:, t * D_NBR + j:t * D_NBR + j + 1],
                            axis=0))
                hb = hg2[:].bitcast(BF).rearrange("p (d r) -> p d r", r=R2)
                e2 = ap_.tile([128, D_NBR * K2], BF, name=f"e2{t}", tag="e2")
                nc.vector.tensor_tensor(
                    out=e2[:].rearrange("p (d k) -> p d k", k=K2),
                    in0=hb[:, :, H2 + 8:H2 + 16],
                    in1=t2rows[:, t * R2 + H2:t * R2 + H2 + 8].unsqueeze(1)
                        .to_broadcast([128, D_NBR, K2]),
                    op=OP.add)
                u2 = ap_.tile([128, D_NBR * K2], BF, name=f"u2{t}", tag="u2")
                nc.scalar.activation(out=u2[:], in_=e2[:], func=AF.Lrelu,
                                     alpha=NEG_SLOPE)
                nc.scalar.activation(out=u2[:], in_=u2[:], func=AF.Exp)
                z2 = ap_.tile([128, K2], FP, name=f"z2{t}", tag="z2")
                nc.vector.tensor_reduce(
                    out=z2[:],
                    in_=u2[:].rearrange("p (d k) -> p d k", k=K2)
                        .transpose([0, 2, 1]),
                    axis=AX.X, op=OP.add)
                rz2 = ap_.tile([128, K2], FP, name=f"rz2{t}", tag="rz2")
                nc.vector.reciprocal(out=rz2[:], in_=z2[:])
                tmp2 = ap_.tile([128, H2 * D_NBR], BF, name=f"t2{t}", tag="t2")
                h24 = hb[:, :, 0:H2].rearrange("p d (k f) -> p d k f", f=F2) \
                    .transpose([0, 2, 3, 1])
                u24 = u2[:].rearrange("p (d k) -> p d k", k=K2).unsqueeze(3) \
                    .to_broadcast([128, D_NBR, K2, F2]).transpose([0, 2, 3, 1])
                nc.vector.tensor_tensor(
                    out=tmp2[:].rearrange("p (k f d) -> p k f d",
                                          f=F2, d=D_NBR),
                    in0=h24, in1=u24, op=OP.mult)
                s2t = ap_.tile([128, H2], FP, name=f"s2{t}", tag="s2")
                nc.vector.tensor_reduce(
                    out=s2t[:],
                    in_=tmp2[:].rearrange("p (kf d) -> p kf d", d=D_NBR),
                    axis=AX.X, op=OP.add)
                o2 = ap_.tile([128, H2], FP, name=f"o2{t}", tag="o2")
                nc.vector.tensor_tensor(
                    out=o2[:].rearrange("p (k f) -> p k f", f=F2),
                    in0=s2t[:].rearrange("p (k f) -> p k f", f=F2),
                    in1=rz2[:].unsqueeze(2).to_broadcast([128, K2, F2]),
                    op=OP.mult)
                mo = ap_.tile([128, F2], FP, name=f"mo{t}", tag="mo")
                nc.vector.tensor_reduce(
                    out=mo[:],
                    in_=o2[:].rearrange("p (k f) -> p k f", f=F2)
                        .transpose([0, 2, 1]),
                    axis=AX.X, op=OP.add)
                u3 = ap_.tile([128, F2], FP, name=f"u3{t}", tag="u3")
                z3 = ap_.tile([128, 1], FP, name=f"z3{t}", tag="z3")
                nc.scalar.activation(out=u3[:], in_=mo[:], func=AF.Exp,
                                     scale=1.0 / K2, accum_out=z3[:])
                rz3 = ap_.tile([128, 1], FP, name=f"rz3{t}", tag="rz3")
                nc.vector.reciprocal(out=rz3[:], in_=z3[:])
                nc.vector.tensor_tensor(
                    out=outacc[:, t * F2:(t + 1) * F2], in0=u3[:],
                    in1=rz3[:].to_broadcast([128, F2]), op=OP.mult)

            nc.sync.dma_start(out=out[:, :], in_=outacc[:])

    nc.finalize()
    return nc


class _SpmdRunner:
    """jit-once SPMD executor over the 8 axon NeuronCores."""

    def __init__(self, nc, n_cores):
        install_neuronx_cc_hook()
        self.nc, self.n_cores = nc, n_cores
        partition_name = (nc.partition_id_tensor.name
                          if nc.partition_id_tensor else None)
        in_names, out_names, out_avals, zero_outs = [], [], [], []
        for alloc in nc.m.functions[0].allocations:
            if not isinstance(alloc, mybir.MemoryLocationSet):
                continue
            name = alloc.memorylocations[0].name
            if alloc.kind == "ExternalInput":
                if name != partition_name:
                    in_names.append(name)
            elif alloc.kind == "ExternalOutput":
                out_names.append(name)
                shape = tuple(alloc.tensor_shape)
                dtype = mybir.dt.np(alloc.dtype)
                out_avals.append(jax.core.ShapedArray(shape, dtype))
                zero_outs.append(np.zeros(shape, dtype))
        self.in_names, self.out_names = in_names, out_names
        self.out_avals, self.zero_outs = out_avals, zero_outs
        all_in_names = in_names + out_names
        if partition_name is not None:
            all_in_names.append(partition_name)

        def _body(*args):
            operands = list(args)
            if partition_name is not None:
                operands.append(partition_id_tensor())
            return tuple(_bass_exec_p.bind(
                *operands, out_avals=tuple(out_avals),
                in_names=tuple(all_in_names), out_names=tuple(out_names),
                lowering_input_output_aliases=(),
                sim_require_finite=True, sim_require_nnan=True, nc=nc))

        devices = jax.devices()[:n_cores]
        self.mesh = Mesh(np.asarray(devices), ("core",))
        n_params, n_outs = len(in_names), len(out_avals)
        in_specs = (PartitionSpec("core"),) * (n_params + n_outs)
        out_specs = (PartitionSpec("core"),) * n_outs
        self.fn = jax.jit(
            shard_map(_body, mesh=self.mesh, in_specs=in_specs,
                      out_specs=out_specs, check_rep=False),
            keep_unused=True)
        self.sharding = jax.sharding.NamedSharding(self.mesh,
                                                   PartitionSpec("core"))

    def run(self, in_maps):
        per_core = [[np.asarray(m[n]) for n in self.in_names] for m in in_maps]
        concat = [np.concatenate([per_core[c][i] for c in range(self.n_cores)],
                                 axis=0) for i in range(len(self.in_names))]
        zeros = [np.zeros((self.n_cores * z.shape[0], *z.shape[1:]), z.dtype)
                 for z in self.zero_outs]
        dev = [jax.device_put(a, self.sharding) for a in concat + zeros]
        outs = self.fn(*dev)
        jax.block_until_ready(outs)
        res = []
        for c in range(self.n_cores):
            res.append({name: np.asarray(outs[i]).reshape(
                self.n_cores, *self.out_avals[i].shape)[c]
                for i, name in enumerate(self.out_names)})
        return res


def _host_prep(node_features, neighbors, W1, a1_1, a2_1, W2, a1_2, a2_2):
    def blk(a, k, f):
        A = np.zeros((k * f, k), np.float32)
        for kk in range(k):
            A[kk * f:(kk + 1) * f, kk] = a[kk]
        return A

    rt1 = np.concatenate(
        [W1, W1 @ blk(a1_1, K1, F1), W1 @ blk(a2_1, K1, F1)],
        axis=1).astype(ml_dtypes.bfloat16)
    rt2 = np.concatenate(
        [W2, W2 @ blk(a1_2, K2, F2), W2 @ blk(a2_2, K2, F2)],
        axis=1).astype(ml_dtypes.bfloat16)
    x_bf = node_features.astype(ml_dtypes.bfloat16)
    nbr = neighbors.astype(np.int64)

    # map a global node id to its padded table-row id (r*SP + p*NT + t)
    g = np.arange(N_NODES, dtype=np.int64)
    r_, w_ = g // S, g % S
    rowmap = (r_ * SP + (w_ % 128) * NT + w_ // 128).astype(np.int32)

    in_maps = []
    for r in range(N_CORES):
        own = np.arange(r * S, (r + 1) * S, dtype=np.int64)
        own = np.concatenate([own, np.full(SP - S, r * S, np.int64)])
        own_pt = own.reshape(NT, 128)                    # [t, p]
        nb = nbr[own]                                    # [SP, 32]
        nb_pt = nb.reshape(NT, 128, D_NBR)               # [t, p, j]
        # edge stream: per tile, columns (d*128+p) then own (p)
        eidx = np.concatenate(
            [nb_pt.transpose(0, 2, 1),                   # [t, d, p]
             own_pt[:, None, :]], axis=1)                # [t, 33, p]
        xeT = np.ascontiguousarray(
            x_bf[eidx.reshape(-1)].T)                    # [128, NT*EC]
        idx2 = np.ascontiguousarray(
            rowmap[nb_pt].transpose(1, 0, 2)             # [p, t, j]
            .reshape(128, NT * D_NBR))
        in_maps.append({'xe': xeT, 'rt1': rt1, 'rt2': rt2, 'idx2': idx2})
    return in_maps


_RUNNER = None


def _get_runner():
    global _RUNNER
    if _RUNNER is None:
        nc = _build_gat()
        _RUNNER = _SpmdRunner(nc, N_CORES)
    return _RUNNER


def kernel(node_features, neighbors, W1, a1_1, a2_1, W2, a1_2, a2_2):
    node_features = np.asarray(node_features, dtype=np.float32)
    runner = _get_runner()
    in_maps = _host_prep(node_features, np.asarray(neighbors),
                         np.asarray(W1, np.float32),
                         np.asarray(a1_1, np.float32),
                         np.asarray(a2_1, np.float32),
                         np.asarray(W2, np.float32),
                         np.asarray(a1_2, np.float32),
                         np.asarray(a2_2, np.float32))
    res = runner.run(in_maps)
    parts = []
    for c in range(N_CORES):
        o = res[c]['out'].reshape(128, NT, F2).transpose(1, 0, 2)
        parts.append(o.reshape(SP, F2)[:S])
    return np.concatenate(parts, axis=0).astype(np.float32)


# revision 8
# speedup vs baseline: 2.9098x; 1.9640x over previous
"""2-layer GAT (100000 nodes, 32 neighbors) on 8 trn2 NeuronCores.

Strategy (SPMD, one Bass program for all 8 cores; nodes sharded 8 ways):
  - Layer 1 needs NO on-device gather: neighbor indices are static inputs,
    so the host pre-expands node features into edge order (one 128-column
    block per destination tile and neighbor slot, plus the tile's own
    columns).  Each tile then runs 33 matmuls against a fused rhs
    [W1 | W1@A1blk | W1@A2blk] emitting h1/s1/s2 per edge directly.
  - Attention (leaky_relu -> exp -> softmax -> weighted sum) on DVE/ACT in
    node-per-partition layout; ELU; PE-transpose feeds the layer-2 matmul
    against [W2 | W2@A1blk2 | W2@A2blk2] per tile, producing each core's
    shard of the layer-2 node table [h2(128)|s1(8)|s2(8)] in bf16.
  - Table shards are AllGathered (28.9 MB) so every core sees all rows;
    layer-2 neighbor rows are fetched with per-slot indirect DMAs
    (one [128,1]-offset gather per neighbor slot per tile).
  - Layer-2 attention, head-mean and softmax produce the output shard.
Host remaps neighbor ids to the partition-major table layout and
de-permutes the output shard.
"""
import sys

if '/opt/trn_rl_repo' not in sys.path:
    sys.path.insert(0, '/opt/trn_rl_repo')

import numpy as np
import ml_dtypes
import concourse.bass as bass
import concourse.bacc as bacc
import concourse.mybir as mybir
from concourse.tile import TileContext
from concourse.masks import make_identity

import jax
from jax.sharding import Mesh, PartitionSpec
from jax.experimental.shard_map import shard_map
from concourse.bass2jax import (_bass_exec_p, install_neuronx_cc_hook,
                                partition_id_tensor)

FP = mybir.dt.float32
BF = mybir.dt.bfloat16
I32 = mybir.dt.int32
AF = mybir.ActivationFunctionType
OP = mybir.AluOpType
AX = mybir.AxisListType

N_NODES = 100000
N_CORES = 8
D_NBR = 32
K1, F1 = 8, 8
K2, F2 = 8, 16
NEG_SLOPE = 0.01
S = N_NODES // N_CORES          # 12500
NT = (S + 127) // 128           # 98 tiles
SP = NT * 128                   # 12544 padded shard rows
H1, H2 = K1 * F1, K2 * F2       # 64, 128
R2 = H2 + 16                    # 144 bf16 = 288B table-2 row
R2F = R2 // 2                   # 72 f32 (fp32-typed view for the gather)
EC = (D_NBR + 1) * 128          # 4224 edge+own columns per tile


def _build_gat():
    nc = bacc.Bacc("TRN2", target_bir_lowering=False, debug=False,
                   num_devices=N_CORES)
    xe = nc.dram_tensor("xe", [128, NT * EC], BF, kind="ExternalInput").ap()
    rt1d = nc.dram_tensor("rt1", [128, H1 + 16], BF, kind="ExternalInput").ap()
    rt2d = nc.dram_tensor("rt2", [H1, R2], BF, kind="ExternalInput").ap()
    idx2 = nc.dram_tensor("idx2", [128, NT * D_NBR], I32,
                          kind="ExternalInput").ap()
    out = nc.dram_tensor("out", [128, NT * F2], FP, kind="ExternalOutput").ap()

    t2m = nc.dram_tensor("t2m", [SP, R2F], FP).ap()
    t2f = nc.dram_tensor("t2f", [N_CORES * SP, R2F], FP,
                         addr_space="Shared").ap()

    with TileContext(nc) as tc:
        with tc.tile_pool(name="const", bufs=1) as cpool, \
             tc.tile_pool(name="xep", bufs=3) as xep, \
             tc.tile_pool(name="att", bufs=2) as ap_, \
             tc.tile_pool(name="g2p", bufs=6) as g2p, \
             tc.tile_pool(name="ph", bufs=4, space="PSUM") as php, \
             tc.tile_pool(name="pmisc", bufs=2, space="PSUM") as pmp, \
             tc.tile_pool(name="ptr", bufs=1, space="PSUM") as ptp, \
             tc.tile_pool(name="pt2", bufs=1, space="PSUM") as pt2p:

            rt1 = cpool.tile([128, H1 + 16], BF)
            nc.sync.dma_start(out=rt1[:], in_=rt1d[:, :])
            rt2 = cpool.tile([H1, R2], BF)
            nc.sync.dma_start(out=rt2[:], in_=rt2d[:, :])
            ident = cpool.tile([128, 128], BF)
            make_identity(nc, ident[:])
            idxs = cpool.tile([128, NT * D_NBR], I32)
            nc.sync.dma_start(out=idxs[:], in_=idx2[:, :])
            t2rows = cpool.tile([128, NT * R2], BF)
            outacc = cpool.tile([128, NT * F2], FP)

            # ---------------- layer 1 (edge-streamed) + layer-2 table ----
            for t in range(NT):
                xet = xep.tile([128, EC], BF, name=f"xe{t}", tag="xe")
                nc.sync.dma_start(out=xet[:], in_=xe[:, t * EC:(t + 1) * EC])

                hg = ap_.tile([128, D_NBR * (H1 + 16)], BF,
                              name=f"hg{t}", tag="hg")
                hgv = hg[:].rearrange("p (d r) -> p d r", r=H1 + 16)
                for g in range(8):           # 8 groups of 4 neighbor slots
                    ps = php.tile([128, 512], FP, name=f"ph{t}_{g}", tag="ph",
                                  space="PSUM")
                    for s in range(4):
                        d = g * 4 + s
                        nc.tensor.matmul(
                            out=ps[:, s * 128:s * 128 + H1 + 16],
                            lhsT=xet[:, d * 128:(d + 1) * 128], rhs=rt1[:],
                            start=True, stop=True)
                    pv = ps[:].rearrange("p (s r) -> p s r", r=128)
                    if g % 2 == 0:
                        nc.vector.tensor_copy(
                            out=hgv[:, g * 4:(g + 1) * 4, :],
                            in_=pv[:, :, :H1 + 16])
                    else:
                        nc.scalar.activation(
                            out=hgv[:, g * 4:(g + 1) * 4, :],
                            in_=pv[:, :, :H1 + 16], func=AF.Copy)
                pown = pmp.tile([128, H1 + 16], FP, name=f"po{t}", tag="po",
                                space="PSUM")
                nc.tensor.matmul(out=pown[:], lhsT=xet[:, D_NBR * 128:],
                                 rhs=rt1[:], start=True, stop=True)
                s1t = ap_.tile([128, K1], BF, name=f"s1{t}", tag="s1")
                nc.vector.tensor_copy(out=s1t[:], in_=pown[:, H1:H1 + 8])

                # attention scores: e = s1_i + s2_j ; exp(leaky_relu(e))
                e = ap_.tile([128, D_NBR * K1], BF, name=f"e{t}", tag="e")
                nc.vector.tensor_tensor(
                    out=e[:].rearrange("p (d k) -> p d k", k=K1),
                    in0=hgv[:, :, H1 + 8:H1 + 16],
                    in1=s1t[:].unsqueeze(1).to_broadcast([128, D_NBR, K1]),
                    op=OP.add)
                u = ap_.tile([128, D_NBR * K1], BF, name=f"u{t}", tag="u")
                nc.scalar.activation(out=u[:], in_=e[:], func=AF.Lrelu,
                                     alpha=NEG_SLOPE)
                nc.scalar.activation(out=u[:], in_=u[:], func=AF.Exp)
                z = ap_.tile([128, K1], FP, name=f"z{t}", tag="z")
                nc.vector.tensor_reduce(
                    out=z[:],
                    in_=u[:].rearrange("p (d k) -> p d k", k=K1)
                        .transpose([0, 2, 1]),
                    axis=AX.X, op=OP.add)
                rz = ap_.tile([128, K1], FP, name=f"rz{t}", tag="rz")
                nc.vector.reciprocal(out=rz[:], in_=z[:])
                tmp = ap_.tile([128, H1 * D_NBR], BF, name=f"tm{t}", tag="tm")
                h4 = hgv[:, :, 0:H1].rearrange("p d (k f) -> p d k f", f=F1) \
                    .transpose([0, 2, 3, 1])
                u4 = u[:].rearrange("p (d k) -> p d k", k=K1).unsqueeze(3) \
                    .to_broadcast([128, D_NBR, K1, F1]).transpose([0, 2, 3, 1])
                nc.vector.tensor_tensor(
                    out=tmp[:].rearrange("p (k f d) -> p k f d",
                                         f=F1, d=D_NBR),
                    in0=h4, in1=u4, op=OP.mult)
                sw = ap_.tile([128, H1], FP, name=f"sw{t}", tag="sw")
                nc.vector.tensor_reduce(
                    out=sw[:],
                    in_=tmp[:].rearrange("p (kf d) -> p kf d", d=D_NBR),
                    axis=AX.X, op=OP.add)
                o = ap_.tile([128, H1], FP, name=f"o{t}", tag="o")
                nc.vector.tensor_tensor(
                    out=o[:].rearrange("p (k f) -> p k f", f=F1),
                    in0=sw[:].rearrange("p (k f) -> p k f", f=F1),
                    in1=rz[:].unsqueeze(2).to_broadcast([128, K1, F1]),
                    op=OP.mult)
                # elu(o) = max(o, exp(min(o,0)) - 1), cast to bf16
                mn = ap_.tile([128, H1], FP, name=f"mn{t}", tag="mn")
                nc.vector.tensor_scalar_min(out=mn[:], in0=o[:], scalar1=0.0)
                nc.scalar.activation(out=mn[:], in_=mn[:], func=AF.Exp)
                x2 = ap_.tile([128, H1], BF, name=f"x2{t}", tag="x2")
                nc.vector.scalar_tensor_tensor(
                    out=x2[:], in0=mn[:], scalar=-1.0, in1=o[:],
                    op0=OP.add, op1=OP.max)
                # layer-2 table rows for own nodes
                ptr = ptp.tile([H1, 128], BF, name=f"pt{t}", tag="pt",
                               space="PSUM")
                nc.tensor.transpose(out=ptr[:], in_=x2[:], identity=ident[:])
                x2T = ap_.tile([H1, 128], BF, name=f"xt{t}", tag="xt")
                nc.scalar.activation(out=x2T[:], in_=ptr[:], func=AF.Copy)
                pt2 = pt2p.tile([128, R2], FP, name=f"p2{t}", tag="p2",
                                space="PSUM")
                nc.tensor.matmul(out=pt2[:], lhsT=x2T[:], rhs=rt2[:],
                                 start=True, stop=True)
                nc.vector.tensor_copy(
                    out=t2rows[:, t * R2:(t + 1) * R2], in_=pt2[:])

            # store shard (partition-major rows: row = p*NT + t) + AllGather
            nc.sync.dma_start(
                out=t2m[:, :].rearrange("(p t) r -> p (t r)", p=128),
                in_=t2rows[:].bitcast(FP))
            nc.gpsimd.collective_compute(
                "AllGather", OP.bypass,
                replica_groups=[list(range(N_CORES))],
                ins=[t2m.opt()], outs=[t2f.opt()])

            # ---------------- layer 2 attention ---------------------------
            for t in range(NT):
                hg2 = g2p.tile([128, D_NBR * R2F], FP, name=f"g2{t}", tag="g2")
                hv = hg2[:].rearrange("p (d r) -> p d r", r=R2F)
                for j in range(D_NBR):
                    nc.gpsimd.indirect_dma_start(
                        out=hv[:, j, :], out_offset=None, in_=t2f[:],
                        in_offset=bass.IndirectOffsetOnAxis(
                            ap=idxs[:, t * D_NBR + j:t * D_NBR + j + 1],
                            axis=0))
                hb = hg2[:].bitcast(BF).rearrange("p (d r) -> p d r", r=R2)
                e2 = ap_.tile([128, D_NBR * K2], BF, name=f"e2{t}", tag="e2")
                nc.vector.tensor_tensor(
                    out=e2[:].rearrange("p (d k) -> p d k", k=K2),
                    in0=hb[:, :, H2 + 8:H2 + 16],
                    in1=t2rows[:, t * R2 + H2:t * R2 + H2 + 8].unsqueeze(1)
                        .to_broadcast([128, D_NBR, K2]),
                    op=OP.add)
                u2 = ap_.tile([128, D_NBR * K2], BF, name=f"u2{t}", tag="u2")
                nc.scalar.activation(out=u2[:], in_=e2[:], func=AF.Lrelu,
                                     alpha=NEG_SLOPE)
                nc.scalar.activation(out=u2[:], in_=u2[:], func=AF.Exp)
                z2 = ap_.tile([128, K2], FP, name=f"z2{t}", tag="z2")
                nc.vector.tensor_reduce(
                    out=z2[:],
                    in_=u2[:].rearrange("p (d k) -> p d k", k=K2)
                        .transpose([0, 2, 1]),
                    axis=AX.X, op=OP.add)
                rz2 = ap_.tile([128, K2], FP, name=f"rz2{t}", tag="rz2")
                nc.vector.reciprocal(out=rz2[:], in_=z2[:])
                tmp2 = ap_.tile([128, H2 * D_NBR], BF, name=f"t2{t}", tag="t2")
                h24 = hb[:, :, 0:H2].rearrange("p d (k f) -> p d k f", f=F2) \
                    .transpose([0, 2, 3, 1])
                u24 = u2[:].rearrange("p (d k) -> p d k", k=K2).unsqueeze(3) \
                    .to_broadcast([128, D_NBR, K2, F2]).transpose([0, 2, 3, 1])
                nc.vector.tensor_tensor(
                    out=tmp2[:].rearrange("p (k f d) -> p k f d",
                                          f=F2, d=D_NBR),
                    in0=h24, in1=u24, op=OP.mult)
                s2t = ap_.tile([128, H2], FP, name=f"s2{t}", tag="s2")
                nc.vector.tensor_reduce(
                    out=s2t[:],
                    in_=tmp2[:].rearrange("p (kf d) -> p kf d", d=D_NBR),
                    axis=AX.X, op=OP.add)
                o2 = ap_.tile([128, H2], FP, name=f"o2{t}", tag="o2")
                nc.vector.tensor_tensor(
                    out=o2[:].rearrange("p (k f) -> p k f", f=F2),
                    in0=s2t[:].rearrange("p (k f) -> p k f", f=F2),
                    in1=rz2[:].unsqueeze(2).to_broadcast([128, K2, F2]),
                    op=OP.mult)
                mo = ap_.tile([128, F2], FP, name=f"mo{t}", tag="mo")
                nc.vector.tensor_reduce(
                    out=mo[:],
                    in_=o2[:].rearrange("p (k f) -> p k f", f=F2)
                        .transpose([0, 2, 1]),
                    axis=AX.X, op=OP.add)
                u3 = ap_.tile([128, F2], FP, name=f"u3{t}", tag="u3")
                z3 = ap_.tile([128, 1], FP, name=f"z3{t}", tag="z3")
                nc.scalar.activation(out=u3[:], in_=mo[:], func=AF.Exp,
                                     scale=1.0 / K2, accum_out=z3[:])
                rz3 = ap_.tile([128, 1], FP, name=f"rz3{t}", tag="rz3")
                nc.vector.reciprocal(out=rz3[:], in_=z3[:])
                nc.vector.tensor_tensor(
                    out=outacc[:, t * F2:(t + 1) * F2], in0=u3[:],
                    in1=rz3[:].to_broadcast([128, F2]), op=OP.mult)

            nc.sync.dma_start(out=out[:, :], in_=outacc[:])

    nc.finalize()
    return nc


class _SpmdRunner:
    """jit-once SPMD executor over the 8 axon NeuronCores."""

    def __init__(self, nc, n_cores):
        install_neuronx_cc_hook()
        self.nc, self.n_cores = nc, n_cores
        partition_name = (nc.partition_id_tensor.name
                          if nc.partition_id_tensor else None)
        in_names, out_names, out_avals, zero_outs = [], [], [], []
        for alloc in nc.m.functions[0].allocations:
            if not isinstance(alloc, mybir.MemoryLocationSet):
                continue
            name = alloc.memorylocations[0].name
            if alloc.kind == "ExternalInput":
                if name != partition_name:
                    in_names.append(name)
            elif alloc.kind == "ExternalOutput":
                out_names.append(name)
                shape = tuple(alloc.tensor_shape)
                dtype = mybir.dt.np(alloc.dtype)
                out_avals.append(jax.core.ShapedArray(shape, dtype))
                zero_outs.append(np.zeros(shape, dtype))
        self.in_names, self.out_names = in_names, out_names
        self.out_avals, self.zero_outs = out_avals, zero_outs
        all_in_names = in_names + out_names
        if partition_name is not None:
            all_in_names.append(partition_name)

        def _body(*args):
            operands = list(args)
            if partition_name is not None:
                operands.append(partition_id_tensor())
            return tuple(_bass_exec_p.bind(
                *operands, out_avals=tuple(out_avals),
                in_names=tuple(all_in_names), out_names=tuple(out_names),
                lowering_input_output_aliases=(),
                sim_require_finite=True, sim_require_nnan=True, nc=nc))

        devices = jax.devices()[:n_cores]
        self.mesh = Mesh(np.asarray(devices), ("core",))
        n_params, n_outs = len(in_names), len(out_avals)
        in_specs = (PartitionSpec("core"),) * (n_params + n_outs)
        out_specs = (PartitionSpec("core"),) * n_outs
        self.fn = jax.jit(
            shard_map(_body, mesh=self.mesh, in_specs=in_specs,
                      out_specs=out_specs, check_rep=False),
            keep_unused=True)
        self.sharding = jax.sharding.NamedSharding(self.mesh,
                                                   PartitionSpec("core"))

    def run(self, in_maps):
        per_core = [[np.asarray(m[n]) for n in self.in_names] for m in in_maps]
        concat = [np.concatenate([per_core[c][i] for c in range(self.n_cores)],
                                 axis=0) for i in range(len(self.in_names))]
        zeros = [np.zeros((self.n_cores * z.shape[0], *z.shape[1:]), z.dtype)
                 for z in self.zero_outs]
        dev = [jax.device_put(a, self.sharding) for a in concat + zeros]
        outs = self.fn(*dev)
        jax.block_until_ready(outs)
        res = []
        for c in range(self.n_cores):
            res.append({name: np.asarray(outs[i]).reshape(
                self.n_cores, *self.out_avals[i].shape)[c]
                for i, name in enumerate(self.out_names)})
        return res


def _host_prep(node_features, neighbors, W1, a1_1, a2_1, W2, a1_2, a2_2):
    def blk(a, k, f):
        A = np.zeros((k * f, k), np.float32)
        for kk in range(k):
            A[kk * f:(kk + 1) * f, kk] = a[kk]
        return A

    rt1 = np.concatenate(
        [W1, W1 @ blk(a1_1, K1, F1), W1 @ blk(a2_1, K1, F1)],
        axis=1).astype(ml_dtypes.bfloat16)
    rt2 = np.concatenate(
        [W2, W2 @ blk(a1_2, K2, F2), W2 @ blk(a2_2, K2, F2)],
        axis=1).astype(ml_dtypes.bfloat16)
    x_bf = node_features.astype(ml_dtypes.bfloat16)
    nbr = neighbors.astype(np.int64)

    # map a global node id to its padded table-row id (r*SP + p*NT + t)
    g = np.arange(N_NODES, dtype=np.int64)
    r_, w_ = g // S, g % S
    rowmap = (r_ * SP + (w_ % 128) * NT + w_ // 128).astype(np.int32)

    in_maps = []
    for r in range(N_CORES):
        own = np.arange(r * S, (r + 1) * S, dtype=np.int64)
        own = np.concatenate([own, np.full(SP - S, r * S, np.int64)])
        own_pt = own.reshape(NT, 128)                    # [t, p]
        nb = nbr[own]                                    # [SP, 32]
        nb_pt = nb.reshape(NT, 128, D_NBR)               # [t, p, j]
        # edge stream: per tile, columns (d*128+p) then own (p)
        eidx = np.concatenate(
            [nb_pt.transpose(0, 2, 1),                   # [t, d, p]
             own_pt[:, None, :]], axis=1)                # [t, 33, p]
        xeT = np.ascontiguousarray(
            x_bf[eidx.reshape(-1)].T)                    # [128, NT*EC]
        idx2 = np.ascontiguousarray(
            rowmap[nb_pt].transpose(1, 0, 2)             # [p, t, j]
            .reshape(128, NT * D_NBR))
        in_maps.append({'xe': xeT, 'rt1': rt1, 'rt2': rt2, 'idx2': idx2})
    return in_maps


_RUNNER = None


def _get_runner():
    global _RUNNER
    if _RUNNER is None:
        nc = _build_gat()
        _RUNNER = _SpmdRunner(nc, N_CORES)
    return _RUNNER


def kernel(node_features, neighbors, W1, a1_1, a2_1, W2, a1_2, a2_2):
    node_features = np.asarray(node_features, dtype=np.float32)
    runner = _get_runner()
    in_maps = _host_prep(node_features, np.asarray(neighbors),
                         np.asarray(W1, np.float32),
                         np.asarray(a1_1, np.float32),
                         np.asarray(a2_1, np.float32),
                         np.asarray(W2, np.float32),
                         np.asarray(a1_2, np.float32),
                         np.asarray(a2_2, np.float32))
    res = runner.run(in_maps)
    parts = []
    for c in range(N_CORES):
        o = res[c]['out'].reshape(128, NT, F2).transpose(1, 0, 2)
        parts.append(o.reshape(SP, F2)[:S])
    return np.concatenate(parts, axis=0).astype(np.float32)
